# revision 30
# baseline (speedup 1.0000x reference)
"""Grouped Conv2D (G=8, 3x3, SAME) on 8 TRN2 NeuronCores via Bass/Tile.

Sharding: data-parallel over batch (32 images -> 4 per core).

Quadrant (space-to-depth) scheme, uniform for all 8 groups: SBUF
partitions hold (hp, wp, ci) -- the 4 pixels of a stride-2 2x2 input
block for one group -- with zero input duplication. PSUM partitions
pack (hq, wq, co): the 2x2 output block. Taps decompose over moving
shifts (dh, dw) in {0,1}^2: kh = 2*dh + hp - hq, kw = 2*dw + wp - wq,
each (tap, output) pair covered exactly once.

Compute runs in fp8 (e4m3) DoubleRow matmuls at 2x PE rate, with
error compensation: x = xh + xl (fp8 value + fp8 residual), W split
likewise (scaled by 16 for subnormal headroom; host divides the
output by 16). y = Wh*xh + Wh*xl + Wl*xh leaves only O(eps^2)
error (~3e-3 relative, same as a plain bf16 kernel). DoubleRow
contracts a leading pair dim on both operands (out = sum_i
W[:,i].T @ X[:,i]); the pair dim carries the two dh shifts as an
aliased-stride view of the input tile, so per PSUM block the whole
sum is 6 half-rate matmuls (3 terms x 2 dw) instead of 8.

The kernel is DMA-bound (~39us of HBM traffic vs ~31us of PE), so
the schedule holds the serial DMA engine stream: weight DMAs ride
the gpsimd SWDGE lane (the SP/HWDGE lane's ~650ns-per-DMA issue
pipeline binds the head), input tiles stream per-group with group 0
split finely, and output DMAs sit after all input DMAs in SP
program order so their transfer requests queue behind every input's.
"""

import numpy as np
import ml_dtypes

import concourse.bass as bass
import concourse.mybir as mybir
import concourse.tile as tile
from concourse.ap import AP
from concourse.bass_utils import run_bass_kernel_spmd
from concourse.vector_clock import ScopedClock

# Problem constants (hardcoded per harness contract).
B, H, W, C = 32, 56, 56, 256
G = 8
CPG = C // G  # 32
KH = KW = 3
NCORES = 8
BC = B // NCORES  # 4 batches per core
HB = 29  # padded stride-2 tile extent (rows -1..56 -> 29 pairs)
WSCALE = 16.0  # weight pre-scale; host divides the output by it

_F32 = mybir.dt.float32
_BF16 = mybir.dt.bfloat16
_FP8 = mybir.dt.float8e4
_BF16NP = np.dtype(ml_dtypes.bfloat16)
_FP8NP = np.dtype(ml_dtypes.float8_e4m3)
_DR = mybir.MatmulPerfMode.DoubleRow


def _max_waits(inst):
    # This container's walrus rejects instructions carrying several sync
    # waits ("Too many sync wait commands"); matmul lowers through the
    # LDWEIGHTS struct which is strictest, and the SP drain's NO_STRUCT
    # encoding also rejects them, so give those zero embedded waits.
    if isinstance(inst, (mybir.InstMatmult, mybir.InstDrain)):
        return 0
    return 1


def _split_sync_waits(nc):
    """Hoist excess sync waits onto same-engine nops placed just before
    the owning instruction (program order on one sequencer preserves the
    wait semantics)."""
    cnt = 0
    for bb in nc.m.functions[0].blocks:
        insts = list(bb.instructions)
        if not any(
            inst.sync_info is not None
            and len(inst.sync_info.on_wait) > _max_waits(inst)
            for inst in insts
        ):
            continue
        newl = []
        for inst in insts:
            si = inst.sync_info
            waits = list(si.on_wait) if si is not None else []
            maxw = _max_waits(inst)
            if len(waits) > maxw:
                for wv in waits[maxw:]:
                    cnt += 1
                    nop = mybir.InstNoOp(
                        name=f"waitsplit-{cnt}",
                        engine=inst.engine,
                        bass_nofuse=True,
                        sync_info=mybir.SyncInfo(on_wait=[wv], on_update=[]),
                    )
                    nc.register_instruction(nop, overwrite=True)
                    newl.append(nop)
                inst.sync_info = mybir.SyncInfo(
                    on_wait=waits[:maxw], on_update=list(si.on_update)
                )
            newl.append(inst)
        live = bb.instructions
        live.clear()
        for inst in newl:
            bb.add_instruction(inst)


def _patch_tile_drain():
    if getattr(tile.TileContext, "_drain_patch_applied", False):
        return

    def _drain_and_barrier(self, tick_clock, wait_clock):
        nc = self.nc
        probe = nc.sync.nop(nofuse=True)
        wait_clock.add_sem_waits(
            probe.ins, ScopedClock({None: tick_clock.global_clock})
        )
        nc.sync.drain()
        nc.all_engine_barrier()
        assert self.sems is not None
        popped = nc._tile_sem_poison_stack.pop()
        assert popped is self._sem_poison
        nc.clear_and_free_semaphores(list(self.sems.allocated().values()))
        _split_sync_waits(nc)

    tile.TileContext._drain_and_barrier = _drain_and_barrier
    tile.TileContext._drain_patch_applied = True


def build_bass():
    """One SPMD Bass program; every core runs it on its own batch shard."""
    _patch_tile_drain()
    nc = bass.Bass("TRN2", target_bir_lowering=False, debug=False,
                   num_devices=NCORES)
    # x: [g, (hp*64+wp*32+ci), b, hh, s, ww] with s=0 the fp8 value and
    #    s=1 the fp8 residual of xpad[b, 2hh+hp-1, 2ww+wp-1, 32g+ci].
    # Row-interleaved hi/lo planes keep the moving AP's last dim stride-1
    # (a walrus DoubleRow requirement) and DMA descriptors large.
    x = nc.dram_tensor("x", [G, 128, BC, HB, 2, HB], _FP8,
                       kind="ExternalInput")
    # w: [(hp,wp,ci), g, ty(hi/lo), dw, i(=dh pair), (hq*64+wq*32+co)] =
    #    fp8 split of WSCALE*kern[2i+hp-hq, 2dw+wp-wq, ci, 32g+co]
    w = nc.dram_tensor("w", [128, G, 2, 2, 2, 128], _FP8,
                       kind="ExternalInput")
    # y: [g, (hq*64+wq*32+co), b, cc, h7, t] =
    #    WSCALE * out[b, 2*(14cc+h7)+hq, 2t+wq, 32g+co]
    y = nc.dram_tensor("y", [G, 128, BC, 2, 14, 28], _BF16,
                       kind="ExternalOutput")

    with tile.TileContext(nc) as tc:
        with (
            tc.tile_pool(name="wpool", bufs=1) as wpool,
            tc.tile_pool(name="xpool", bufs=1) as xpool,
            tc.tile_pool(name="ypool", bufs=1) as ypool,
            tc.tile_pool(name="psum", bufs=6, space=bass.MemorySpace.PSUM) as pp,
        ):
            wt = wpool.tile([128, G, 2, 2, 2, 128], _FP8, tag="wt")
            xts = {}
            for g in range(G):
                xts[g] = xpool.tile([128, BC, HB, 2, HB], _FP8, tag=f"x{g}",
                                    name=f"xt_{g}")
            ygs = {}
            for g in range(G):
                ygs[g] = ypool.tile([128, BC, 2, 14, 28], _BF16,
                                    tag=f"y{g}", name=f"yg_{g}")

            # Input DMA stream, two issue lanes. The head is bound by the
            # serial HWDGE descriptor-generation pipeline (~650ns per DMA),
            # so all weight DMAs ride the gpsimd SWDGE lane, which generates
            # descriptors on the Pool engine in parallel; the SP/HWDGE lane
            # carries only the input tiles. Group 0 is split finely (rows
            # 0:15 cover the whole first PSUM block) so compute starts as
            # early as possible, and per-b so the PE never outruns the
            # issue-limited head of the stream.
            # Pool lane: weight slice for group g, then the pad-row memsets
            # for group g+1 (hp=0 partitions never get row hh=0 DMA'd, hp=1
            # never row hh=28 -- both are all-zero SAME padding), so each
            # group's memsets complete well before its input DMA lands.
            nc.gpsimd.dma_start(wt[:, 0], w[:, 0])
            for g in range(1, G):
                nc.gpsimd.dma_start(wt[:, g], w[:, g])
                nc.gpsimd.memset(xts[g][0:64, :, 0], 0)
                nc.gpsimd.memset(xts[g][64:128, :, HB - 1], 0)
            nc.sync.dma_start(xts[0][:, 0, 0:15], x[0, :, 0, 0:15])
            nc.sync.dma_start(xts[0][:, 0, 15:HB], x[0, :, 0, 15:HB])
            nc.sync.dma_start(xts[0][:, 1], x[0, :, 1])
            nc.sync.dma_start(xts[0][:, 2], x[0, :, 2])
            nc.sync.dma_start(xts[0][:, 3], x[0, :, 3])
            for g in range(1, G):
                nc.sync.dma_start(xts[g][0:64, :, 1:HB], x[g, 0:64, :, 1:HB])
                nc.sync.dma_start(xts[g][64:128, :, 0:HB - 1],
                                  x[g, 64:128, :, 0:HB - 1])

            # Compute: per (g, b, cc) one PSUM block [128, 14, 28], six
            # DoubleRow matmuls: (Wh,xh), (Wh,xl), (Wl,xh) x dw in {0,1}.
            # The DoubleRow pair dim carries the two dh shifts via an
            # aliased-stride view (pair stride == one hh row == HB*2 elems).
            def moving(g, b, cc, h0, h1, dw, s):
                base = xts[g][:]
                off = b * (HB * 2 * HB) + (14 * cc + h0) * (2 * HB) \
                    + s * HB + dw
                return AP(base.tensor, base.offset + off, [
                    list(base.ap[0]),      # partition dim
                    [2 * HB, 2],           # dh pair (aliases the hh axis)
                    [2 * HB, h1 - h0],     # h' rows
                    [1, 28],               # t columns (contiguous)
                ])

            ci = 0
            for g in range(G):
                for b in range(BC):
                    for cc in range(2):
                        ps = pp.tile([128, 14, 28], _F32, tag="ps")
                        terms = [(0, 0, 0), (0, 1, 0),
                                 (0, 0, 1), (0, 1, 1),
                                 (1, 0, 0), (1, 1, 0)]
                        for i, (ty, dw, s) in enumerate(terms):
                            nc.tensor.matmul(
                                ps[:, :, :],
                                wt[:, g, ty, dw],
                                moving(g, b, cc, 0, 14, dw, s),
                                start=(i == 0),
                                stop=(i == len(terms) - 1),
                                perf_mode=_DR,
                            )
                        dst = ygs[g][:, b, cc]
                        if ci % 2 == 0:
                            nc.vector.tensor_copy(dst, ps[:, :, :])
                        else:
                            nc.scalar.copy(dst, ps[:, :, :])
                        ci += 1

            # Output DMAs: one per group (the kernel is DMA-stream-bound;
            # big transfers keep the serial DMA engines ahead of the
            # ~650ns-per-DMA SP issue pipeline). yg covers [128, BC, 2,
            # 14, 28] contiguously per partition.
            for g in range(G):
                nc.sync.dma_start(y[g], ygs[g][:])
    return nc


_NC_CACHE = None


def _get_nc():
    global _NC_CACHE
    if _NC_CACHE is None:
        _NC_CACHE = build_bass()
    return _NC_CACHE


def _fp8_split(a):
    """fp32 array -> (hi, lo) fp8 e4m3 value + residual."""
    hi = a.astype(_FP8NP)
    lo = (a - hi.astype(np.float32)).astype(_FP8NP)
    return hi, lo


def _pack_x(inputs):
    """[B,H,W,C] fp32 -> [G, 128(hp,wp,ci), B, 29, 29, 2] fp8 quadrants."""
    xpad = np.zeros((B, H + 2, W + 2, C), np.float32)
    xpad[:, 1:H + 1, 1:W + 1, :] = inputs
    s = xpad.strides
    # xv[b, hh, hp, ww, wp, g, ci] = xpad[b, 2hh+hp, 2ww+wp, 32g+ci]
    xv = np.lib.stride_tricks.as_strided(
        xpad, shape=(B, HB, 2, HB, 2, G, CPG),
        strides=(s[0], 2 * s[1], s[1], 2 * s[2], s[2], CPG * s[3], s[3]))
    xt = np.ascontiguousarray(
        xv.transpose(5, 2, 4, 6, 0, 1, 3).reshape(G, 128, B, HB, HB))
    hi, lo = _fp8_split(xt)
    return np.stack([hi, lo], axis=-2)  # [G, 128, B, HB, 2, HB]


def _pack_w(kern):
    """HWIO [3,3,32,256] -> [128(hp,wp,ci), g, ty, dw, i, 128(hq,wq,co)]."""
    wd = np.zeros((128, G, 2, 2, 128), np.float32)
    for dh in range(2):
        for dw in range(2):
            for hp in range(2):
                for hq in range(2):
                    kh = 2 * dh + hp - hq
                    if not 0 <= kh < KH:
                        continue
                    for wp in range(2):
                        for wq in range(2):
                            kw = 2 * dw + wp - wq
                            if not 0 <= kw < KW:
                                continue
                            for g in range(G):
                                wd[hp * 64 + wp * 32:hp * 64 + wp * 32 + 32,
                                   g, dw, dh,
                                   hq * 64 + wq * 32:hq * 64 + wq * 32 + 32] \
                                    = WSCALE * kern[kh, kw, :,
                                                    g * CPG:(g + 1) * CPG]
    hi, lo = _fp8_split(wd)
    return np.stack([hi, lo], axis=2)  # [128, G, ty, dw, i, 128]


def _make_in_maps(inputs, kern):
    inputs = np.asarray(inputs, np.float32)
    kern = np.asarray(kern, np.float32)
    xp = _pack_x(inputs)
    wd = _pack_w(kern)
    return [
        {
            "x": np.ascontiguousarray(xp[:, :, c * BC:(c + 1) * BC]),
            "w": wd,
        }
        for c in range(NCORES)
    ]


def _unpack_y(ya):
    """[G,128,BC,2,14,28] bf16 -> [BC,H,W,C] fp32 (descaled)."""
    o = np.asarray(ya, np.float32).reshape(G, 2, 2, CPG, BC, 2, 14, 28)
    # out[b, 2*(14cc+h7)+hq, 2t+wq, 32g+co]
    out = o.transpose(4, 5, 6, 1, 7, 2, 0, 3).reshape(BC, H, W, C)
    return out * (1.0 / WSCALE)


def kernel(inputs, kernel, bias):
    nc = _get_nc()
    in_maps = _make_in_maps(inputs, kernel)
    try:
        res = run_bass_kernel_spmd(nc, in_maps, list(range(NCORES)))
    except ModuleNotFoundError:
        # BASS_TRACE set but the axon NTFF hook module is absent in this
        # container; retry with tracing suppressed.
        import os

        os.environ["BASS_NEVER_TRACE"] = "1"
        res = run_bass_kernel_spmd(nc, in_maps, list(range(NCORES)))

    outs = [_unpack_y(res.results[c]["y"]) for c in range(NCORES)]
    out = np.concatenate(outs, axis=0)
    out = out + np.asarray(bias, np.float32)
    return out.astype(np.float32)


# revision 35
# speedup vs baseline: 1.0078x; 1.0078x over previous
"""Grouped Conv2D (G=8, 3x3, SAME) on 8 TRN2 NeuronCores via Bass/Tile.

Sharding: data-parallel over batch (32 images -> 4 per core).

Quadrant (space-to-depth) scheme, uniform for all 8 groups: SBUF
partitions hold (hp, wp, ci) -- the 4 pixels of a stride-2 2x2 input
block for one group -- with zero input duplication. PSUM partitions
pack (hq, wq, co): the 2x2 output block. Taps decompose over moving
shifts (dh, dw) in {0,1}^2: kh = 2*dh + hp - hq, kw = 2*dw + wp - wq,
each (tap, output) pair covered exactly once.

Compute runs in fp8 (e4m3) DoubleRow matmuls at 2x PE rate, with
error compensation: x = xh + xl (fp8 value + fp8 residual), W split
likewise (scaled by 16 for subnormal headroom; host divides the
output by 16). y = Wh*xh + Wh*xl + Wl*xh leaves only O(eps^2)
error (~3e-3 relative, same as a plain bf16 kernel). DoubleRow
contracts a leading pair dim on both operands (out = sum_i
W[:,i].T @ X[:,i]); the pair dim carries the two dh shifts as an
aliased-stride view of the input tile, so per PSUM block the whole
sum is 6 half-rate matmuls (3 terms x 2 dw) instead of 8.

The kernel is DMA-bound (~39us of HBM traffic vs ~31us of PE), so
the schedule holds the serial DMA engine stream: weight DMAs ride
the gpsimd SWDGE lane (the SP/HWDGE lane's ~650ns-per-DMA issue
pipeline binds the head), input tiles stream per-group with group 0
split finely, and output DMAs sit after all input DMAs in SP
program order so their transfer requests queue behind every input's.
"""

import numpy as np
import ml_dtypes

import concourse.bass as bass
import concourse.mybir as mybir
import concourse.tile as tile
from concourse.ap import AP
from concourse.bass_utils import run_bass_kernel_spmd
from concourse.vector_clock import ScopedClock

# Problem constants (hardcoded per harness contract).
B, H, W, C = 32, 56, 56, 256
G = 8
CPG = C // G  # 32
KH = KW = 3
NCORES = 8
BC = B // NCORES  # 4 batches per core
HB = 29  # padded stride-2 tile extent (rows -1..56 -> 29 pairs)
WSCALE = 16.0  # weight pre-scale; host divides the output by it

_F32 = mybir.dt.float32
_BF16 = mybir.dt.bfloat16
_FP8 = mybir.dt.float8e4
_BF16NP = np.dtype(ml_dtypes.bfloat16)
_FP8NP = np.dtype(ml_dtypes.float8_e4m3)
_DR = mybir.MatmulPerfMode.DoubleRow


def _max_waits(inst):
    # This container's walrus rejects instructions carrying several sync
    # waits ("Too many sync wait commands"); matmul lowers through the
    # LDWEIGHTS struct which is strictest, and the SP drain's NO_STRUCT
    # encoding also rejects them, so give those zero embedded waits.
    if isinstance(inst, (mybir.InstMatmult, mybir.InstDrain)):
        return 0
    return 1


def _split_sync_waits(nc):
    """Hoist excess sync waits onto same-engine nops placed just before
    the owning instruction (program order on one sequencer preserves the
    wait semantics)."""
    cnt = 0
    for bb in nc.m.functions[0].blocks:
        insts = list(bb.instructions)
        if not any(
            inst.sync_info is not None
            and len(inst.sync_info.on_wait) > _max_waits(inst)
            for inst in insts
        ):
            continue
        newl = []
        for inst in insts:
            si = inst.sync_info
            waits = list(si.on_wait) if si is not None else []
            maxw = _max_waits(inst)
            if len(waits) > maxw:
                for wv in waits[maxw:]:
                    cnt += 1
                    nop = mybir.InstNoOp(
                        name=f"waitsplit-{cnt}",
                        engine=inst.engine,
                        bass_nofuse=True,
                        sync_info=mybir.SyncInfo(on_wait=[wv], on_update=[]),
                    )
                    nc.register_instruction(nop, overwrite=True)
                    newl.append(nop)
                inst.sync_info = mybir.SyncInfo(
                    on_wait=waits[:maxw], on_update=list(si.on_update)
                )
            newl.append(inst)
        live = bb.instructions
        live.clear()
        for inst in newl:
            bb.add_instruction(inst)


def _patch_tile_drain():
    if getattr(tile.TileContext, "_drain_patch_applied", False):
        return

    def _drain_and_barrier(self, tick_clock, wait_clock):
        nc = self.nc
        probe = nc.sync.nop(nofuse=True)
        wait_clock.add_sem_waits(
            probe.ins, ScopedClock({None: tick_clock.global_clock})
        )
        nc.sync.drain()
        nc.all_engine_barrier()
        assert self.sems is not None
        popped = nc._tile_sem_poison_stack.pop()
        assert popped is self._sem_poison
        nc.clear_and_free_semaphores(list(self.sems.allocated().values()))
        _split_sync_waits(nc)

    tile.TileContext._drain_and_barrier = _drain_and_barrier
    tile.TileContext._drain_patch_applied = True


def build_bass():
    """One SPMD Bass program; every core runs it on its own batch shard."""
    _patch_tile_drain()
    nc = bass.Bass("TRN2", target_bir_lowering=False, debug=False,
                   num_devices=NCORES)
    # x: [g, (hp*64+wp*32+ci), b, hh, s, ww] with s=0 the fp8 value and
    #    s=1 the fp8 residual of xpad[b, 2hh+hp-1, 2ww+wp-1, 32g+ci].
    # Row-interleaved hi/lo planes keep the moving AP's last dim stride-1
    # (a walrus DoubleRow requirement) and DMA descriptors large.
    x = nc.dram_tensor("x", [G, 128, BC, HB, 2, HB], _FP8,
                       kind="ExternalInput")
    # w: [(hp,wp,ci), g, ty(hi/lo), dw, i(=dh pair), (hq*64+wq*32+co)] =
    #    fp8 split of WSCALE*kern[2i+hp-hq, 2dw+wp-wq, ci, 32g+co]
    w = nc.dram_tensor("w", [128, G, 2, 2, 2, 128], _FP8,
                       kind="ExternalInput")
    # y: [g, (hq*64+wq*32+co), b, cc, h7, t] =
    #    WSCALE * out[b, 2*(14cc+h7)+hq, 2t+wq, 32g+co]
    y = nc.dram_tensor("y", [G, 128, BC, 2, 14, 28], _BF16,
                       kind="ExternalOutput")

    with tile.TileContext(nc) as tc:
        with (
            tc.tile_pool(name="wpool", bufs=1) as wpool,
            tc.tile_pool(name="xpool", bufs=1) as xpool,
            tc.tile_pool(name="ypool", bufs=1) as ypool,
            tc.tile_pool(name="psum", bufs=6, space=bass.MemorySpace.PSUM) as pp,
        ):
            wt = wpool.tile([128, G, 2, 2, 2, 128], _FP8, tag="wt")
            xts = {}
            for g in range(G):
                xts[g] = xpool.tile([128, BC, HB, 2, HB], _FP8, tag=f"x{g}",
                                    name=f"xt_{g}")
            ygs = {}
            for g in range(G):
                ygs[g] = ypool.tile([128, BC, 2, 14, 28], _BF16,
                                    tag=f"y{g}", name=f"yg_{g}")

            # Input DMA stream, two issue lanes. The head is bound by the
            # serial HWDGE descriptor-generation pipeline (~650ns per DMA),
            # so all weight DMAs ride the gpsimd SWDGE lane, which generates
            # descriptors on the Pool engine in parallel; the SP/HWDGE lane
            # carries only the input tiles. Group 0 is split finely (rows
            # 0:15 cover the whole first PSUM block) so compute starts as
            # early as possible, and per-b so the PE never outruns the
            # issue-limited head of the stream.
            # Pool lane: weight slice for group g, then the pad-row memsets
            # for group g+1 (hp=0 partitions never get row hh=0 DMA'd, hp=1
            # never row hh=28 -- both are all-zero SAME padding), so each
            # group's memsets complete well before its input DMA lands.
            nc.gpsimd.dma_start(wt[:, 0], w[:, 0])
            for g in range(1, G):
                nc.gpsimd.dma_start(wt[:, g], w[:, g])
                nc.gpsimd.memset(xts[g][0:64, :, 0], 0)
                nc.gpsimd.memset(xts[g][64:128, :, HB - 1], 0)
            nc.scalar.dma_start(xts[0][:, 0, 0:15], x[0, :, 0, 0:15])
            nc.scalar.dma_start(xts[0][:, 0, 15:HB], x[0, :, 0, 15:HB])
            nc.sync.dma_start(xts[0][:, 1], x[0, :, 1])
            nc.sync.dma_start(xts[0][:, 2], x[0, :, 2])
            nc.sync.dma_start(xts[0][:, 3], x[0, :, 3])
            for g in range(1, G):
                nc.sync.dma_start(xts[g][0:64, :, 1:HB], x[g, 0:64, :, 1:HB])
                nc.sync.dma_start(xts[g][64:128, :, 0:HB - 1],
                                  x[g, 64:128, :, 0:HB - 1])

            # Compute: per (g, b, cc) one PSUM block [128, 14, 28], six
            # DoubleRow matmuls: (Wh,xh), (Wh,xl), (Wl,xh) x dw in {0,1}.
            # The DoubleRow pair dim carries the two dh shifts via an
            # aliased-stride view (pair stride == one hh row == HB*2 elems).
            def moving(g, b, cc, h0, h1, dw, s):
                base = xts[g][:]
                off = b * (HB * 2 * HB) + (14 * cc + h0) * (2 * HB) \
                    + s * HB + dw
                return AP(base.tensor, base.offset + off, [
                    list(base.ap[0]),      # partition dim
                    [2 * HB, 2],           # dh pair (aliases the hh axis)
                    [2 * HB, h1 - h0],     # h' rows
                    [1, 28],               # t columns (contiguous)
                ])

            ci = 0
            for g in range(G):
                for b in range(BC):
                    for cc in range(2):
                        ps = pp.tile([128, 14, 28], _F32, tag="ps")
                        terms = [(0, 0, 0), (0, 1, 0),
                                 (0, 0, 1), (0, 1, 1),
                                 (1, 0, 0), (1, 1, 0)]
                        for i, (ty, dw, s) in enumerate(terms):
                            nc.tensor.matmul(
                                ps[:, :, :],
                                wt[:, g, ty, dw],
                                moving(g, b, cc, 0, 14, dw, s),
                                start=(i == 0),
                                stop=(i == len(terms) - 1),
                                perf_mode=_DR,
                            )
                        dst = ygs[g][:, b, cc]
                        if ci % 2 == 0:
                            nc.vector.tensor_copy(dst, ps[:, :, :])
                        else:
                            nc.scalar.copy(dst, ps[:, :, :])
                        ci += 1

            # Output DMAs: one per group (the kernel is DMA-stream-bound;
            # big transfers keep the serial DMA engines ahead of the
            # ~650ns-per-DMA SP issue pipeline). yg covers [128, BC, 2,
            # 14, 28] contiguously per partition.
            for g in range(G):
                nc.sync.dma_start(y[g], ygs[g][:])
    return nc


_NC_CACHE = None


def _get_nc():
    global _NC_CACHE
    if _NC_CACHE is None:
        _NC_CACHE = build_bass()
    return _NC_CACHE


def _fp8_split(a):
    """fp32 array -> (hi, lo) fp8 e4m3 value + residual."""
    hi = a.astype(_FP8NP)
    lo = (a - hi.astype(np.float32)).astype(_FP8NP)
    return hi, lo


def _pack_x(inputs):
    """[B,H,W,C] fp32 -> [G, 128(hp,wp,ci), B, 29, 29, 2] fp8 quadrants."""
    xpad = np.zeros((B, H + 2, W + 2, C), np.float32)
    xpad[:, 1:H + 1, 1:W + 1, :] = inputs
    s = xpad.strides
    # xv[b, hh, hp, ww, wp, g, ci] = xpad[b, 2hh+hp, 2ww+wp, 32g+ci]
    xv = np.lib.stride_tricks.as_strided(
        xpad, shape=(B, HB, 2, HB, 2, G, CPG),
        strides=(s[0], 2 * s[1], s[1], 2 * s[2], s[2], CPG * s[3], s[3]))
    xt = np.ascontiguousarray(
        xv.transpose(5, 2, 4, 6, 0, 1, 3).reshape(G, 128, B, HB, HB))
    hi, lo = _fp8_split(xt)
    return np.stack([hi, lo], axis=-2)  # [G, 128, B, HB, 2, HB]


def _pack_w(kern):
    """HWIO [3,3,32,256] -> [128(hp,wp,ci), g, ty, dw, i, 128(hq,wq,co)]."""
    wd = np.zeros((128, G, 2, 2, 128), np.float32)
    for dh in range(2):
        for dw in range(2):
            for hp in range(2):
                for hq in range(2):
                    kh = 2 * dh + hp - hq
                    if not 0 <= kh < KH:
                        continue
                    for wp in range(2):
                        for wq in range(2):
                            kw = 2 * dw + wp - wq
                            if not 0 <= kw < KW:
                                continue
                            for g in range(G):
                                wd[hp * 64 + wp * 32:hp * 64 + wp * 32 + 32,
                                   g, dw, dh,
                                   hq * 64 + wq * 32:hq * 64 + wq * 32 + 32] \
                                    = WSCALE * kern[kh, kw, :,
                                                    g * CPG:(g + 1) * CPG]
    hi, lo = _fp8_split(wd)
    return np.stack([hi, lo], axis=2)  # [128, G, ty, dw, i, 128]


def _make_in_maps(inputs, kern):
    inputs = np.asarray(inputs, np.float32)
    kern = np.asarray(kern, np.float32)
    xp = _pack_x(inputs)
    wd = _pack_w(kern)
    return [
        {
            "x": np.ascontiguousarray(xp[:, :, c * BC:(c + 1) * BC]),
            "w": wd,
        }
        for c in range(NCORES)
    ]


def _unpack_y(ya):
    """[G,128,BC,2,14,28] bf16 -> [BC,H,W,C] fp32 (descaled)."""
    o = np.asarray(ya, np.float32).reshape(G, 2, 2, CPG, BC, 2, 14, 28)
    # out[b, 2*(14cc+h7)+hq, 2t+wq, 32g+co]
    out = o.transpose(4, 5, 6, 1, 7, 2, 0, 3).reshape(BC, H, W, C)
    return out * (1.0 / WSCALE)


def kernel(inputs, kernel, bias):
    nc = _get_nc()
    in_maps = _make_in_maps(inputs, kernel)
    try:
        res = run_bass_kernel_spmd(nc, in_maps, list(range(NCORES)))
    except ModuleNotFoundError:
        # BASS_TRACE set but the axon NTFF hook module is absent in this
        # container; retry with tracing suppressed.
        import os

        os.environ["BASS_NEVER_TRACE"] = "1"
        res = run_bass_kernel_spmd(nc, in_maps, list(range(NCORES)))

    outs = [_unpack_y(res.results[c]["y"]) for c in range(NCORES)]
    out = np.concatenate(outs, axis=0)
    out = out + np.asarray(bias, np.float32)
    return out.astype(np.float32)
